# revision 1
# baseline (speedup 1.0000x reference)
"""Exact entmax-1.5 loss kernel for Trainium2 (8 NeuronCores, data-parallel over rows).

Algorithm (per row of X [N=2048, V=32000] f32):
  The entmax-1.5 threshold tau* solves  sum_j relu(X_j/2 - tau)^2 = 1.
  In X-units (theta = 2*tau):            sum_j relu(X_j - theta)^2 = 4.
  f(theta) is convex decreasing, so Newton iteration from a lower bound
  converges monotonically from below - no sort needed.

  Device pipeline per 128-row block:
    1. Stream X in column chunks into a resident SBUF tile; build 64-wide
       group maxes G2 [128,500] on the fly (DVE).
    2. Newton on f_G2 (a pointwise lower bound of f) gives theta_lb <= theta*
       after 7 cheap iterations (minus a small epsilon).
    3. Candidate 4-element groups per 16-row core: u0 = relu(X - theta_lb)
       (bf16, chunked); 16 accumulating TensorE matmuls with per-residue
       selection matrices produce, directly in "wrapped" [128, 500] layout,
       per-core group activity sums; a group is a union candidate iff > 0.
    4. Per-partition compaction of candidate group ids via one small
       local_scatter; one indirect_copy gathers the 4-float payloads of all
       union groups (core-shared index list). Groups belonging only to other
       rows of the core contribute exactly 0 to every relu sum downstream
       (their values are <= theta_lb), so no masking is needed.
    5. Exact Newton (3 iters) + final stats on the compact [128, 4096]
       buffer: S2f = sum u^2, S3f = sum u^3 with u = relu(X - theta*);
       loss = 4/3 + S3f/12 + theta*.S2f/4 - X[target]; X[target] fetched
       with dma_gather of the 256B block containing it + one-hot dot.

Host wrapper shards rows 256-per-core across 8 cores, no collectives.
"""
import numpy as np
from contextlib import ExitStack

N, V = 2048, 32000
N_CORES = 8
ROWS = N // N_CORES          # 256 rows per core
CHUNK = 1600
NCHUNK = V // CHUNK          # 20
NG4 = V // 4                 # 8000 groups of 4
WF = NG4 // 16               # 500 wrapped columns
G2_ITERS = 7
EXACT_ITERS = 3
STAGE = 99
EPS_LB = 2e-3                # X-units safety margin on the lower bound
S4 = 64                      # per-partition capacity of compacted group ids
KU = 16 * S4                 # 1024 union groups per core
CW = 4 * KU                  # 4096 compact width (f32)
DW = V + 16                  # X tile width with sentinel pad

_nc_cache = {}


def _build_nc():
    import concourse.bass as bass
    import concourse.bacc as bacc
    import concourse.tile as tile
    from concourse import mybir

    f32 = mybir.dt.float32
    bf16 = mybir.dt.bfloat16
    i16 = mybir.dt.int16
    u16 = mybir.dt.uint16
    Alu = mybir.AluOpType
    Act = mybir.ActivationFunctionType
    Ax = mybir.AxisListType

    nc = bacc.Bacc("TRN2", target_bir_lowering=False, debug=False)
    x = nc.dram_tensor("x", [ROWS, V], f32, kind="ExternalInput").ap()
    oh = nc.dram_tensor("oh", [ROWS, 64], f32, kind="ExternalInput").ap()
    tbl = nc.dram_tensor("tbl", [128, 16], i16, kind="ExternalInput").ap()
    iotd = nc.dram_tensor("iot", [128, WF], f32, kind="ExternalInput").ap()
    wseld = nc.dram_tensor("wsel", [128, 16 * 128], bf16, kind="ExternalInput").ap()
    out = nc.dram_tensor("loss", [ROWS], f32, kind="ExternalOutput").ap()

    with tile.TileContext(nc) as tc, ExitStack() as ctx:
        const = ctx.enter_context(tc.tile_pool(name="const", bufs=1))
        big = ctx.enter_context(tc.tile_pool(name="big", bufs=1))
        u0p = ctx.enter_context(tc.tile_pool(name="u0", bufs=2))
        psum = ctx.enter_context(tc.tile_pool(name="psum", bufs=1, space="PSUM"))

        iot = const.tile([128, WF], f32, tag="iot")
        wt = const.tile([128, 16 * 128], bf16, tag="wsel")
        nc.gpsimd.dma_start(iot[:], iotd)
        nc.gpsimd.dma_start(wt[:], wseld)

        for b in range(2):
            xb = x[bass.ts(b, 128), :]

            xt = big.tile([128, DW], f32, tag="xt")
            shared = big.tile([128, 1536], f32, tag="shared")
            small = big.tile([128, 896], f32, tag="small")
            cand = big.tile([128, CW], f32, tag="cand")
            wk = big.tile([128, CW], f32, tag="wk")

            sc = small[:, 0:32]
            m_s = sc[:, 0:1]
            th = sc[:, 1:2]
            nu = sc[:, 2:3]
            S1 = sc[:, 3:4]
            S2 = sc[:, 4:5]
            r1 = sc[:, 5:6]
            dd = sc[:, 6:7]
            S2f = sc[:, 7:8]
            S3f = sc[:, 8:9]
            ta = sc[:, 9:10]
            tb_ = sc[:, 10:11]
            xtg = sc[:, 11:12]
            lo = sc[:, 12:13]
            oh_t = small[:, 32:96]
            gA = small[:, 96:160]
            gB = small[:, 160:224]
            vcomp = small[:, 224:256].bitcast(i16)    # [128, 64] i16
            idxu = small[:, 256:288].bitcast(u16)     # [128, 64] u16
            bneg = small[:, 288:352]                  # [128, 64] f32
            v_i16 = small[:, 352:608].bitcast(i16)    # [128, 512] i16 (use 500)
            rank = small[:, 608:864].bitcast(i16)     # [128, 512] i16 (use 500)
            tbl_t = small[:, 864:872].bitcast(i16)    # [128, 16] i16

            # ---- x[target] gather (independent of the main pipeline) ----
            nc.gpsimd.dma_start(tbl_t[:, 0:16], tbl[:, :])
            nc.gpsimd.dma_start(oh_t, oh[bass.ts(b, 128), :])
            for half, gdst in ((0, gA), (1, gB)):
                c = 2 * b + half
                src = x[64 * c : 64 * (c + 1), :].rearrange(
                    "r (bk e) -> (r bk) e", e=64
                )
                nc.gpsimd.dma_gather(
                    gdst.rearrange("p (one e) -> p one e", one=1),
                    src,
                    tbl_t[:, 4 * c : 4 * (c + 1)],
                    num_idxs=64,
                    num_idxs_reg=64,
                    elem_size=64,
                )
            nc.gpsimd.dma_start(gA[64:128, :], gB[0:64, :])
            nc.vector.tensor_tensor(out=gB[:], in0=gA[:], in1=oh_t[:], op=Alu.mult)
            nc.vector.tensor_reduce(xtg, gB[:], axis=Ax.X, op=Alu.add)

            # ---- stream X + G2 group maxes ----
            nc.vector.memset(xt[:, V:DW], -1e30)
            g2v = shared[:, 0:WF]
            u2 = shared[:, 512 : 512 + WF]
            for c in range(NCHUNK):
                nc.gpsimd.dma_start(xt[:, bass.ts(c, CHUNK)], xb[:, bass.ts(c, CHUNK)])
                nc.vector.tensor_reduce(
                    g2v[:, bass.ts(c, CHUNK // 64)],
                    xt[:, bass.ts(c, CHUNK)].rearrange("p (g k) -> p g k", k=64),
                    axis=Ax.X,
                    op=Alu.max,
                )

            # ---- G2 Newton for the lower bound ----
            nc.vector.tensor_reduce(m_s, g2v, axis=Ax.X, op=Alu.max)
            nc.vector.tensor_scalar(out=th, in0=m_s, scalar1=-2.0, scalar2=None, op0=Alu.add)
            nc.vector.tensor_scalar(out=nu, in0=th, scalar1=-1.0, scalar2=None, op0=Alu.mult)
            for _ in range(G2_ITERS):
                nc.scalar.activation(u2, g2v, Act.Relu, bias=nu, scale=1.0, accum_out=S1)
                nc.scalar.activation(u2, u2, Act.Square, accum_out=S2)
                nc.vector.reciprocal(r1, S1)
                nc.vector.tensor_scalar(
                    out=dd, in0=S2, scalar1=-4.0, scalar2=0.5, op0=Alu.add, op1=Alu.mult
                )
                nc.vector.scalar_tensor_tensor(
                    out=th, in0=dd, scalar=r1, in1=th, op0=Alu.mult, op1=Alu.add
                )
                nc.vector.tensor_scalar(out=nu, in0=th, scalar1=-1.0, scalar2=None, op0=Alu.mult)
            nc.vector.tensor_scalar(out=th, in0=th, scalar1=-EPS_LB, scalar2=None, op0=Alu.add)
            nc.vector.tensor_scalar(out=nu, in0=th, scalar1=-1.0, scalar2=None, op0=Alu.mult)

            if STAGE < 2:
                nc.gpsimd.dma_start(out[bass.ts(b, 128)], th)
                continue
            # ---- candidate counts via relu + residue-selection matmuls ----
            pc = psum.tile([128, NCHUNK, 128], f32, tag="pc")
            for c in range(NCHUNK):
                u0 = u0p.tile([128, CHUNK], bf16, tag="u0")
                if c % 2 == 0:
                    nc.scalar.activation(
                        u0[:], xt[:, bass.ts(c, CHUNK)], Act.Relu, bias=nu, scale=1.0
                    )
                else:
                    nc.vector.tensor_scalar(
                        out=u0[:], in0=xt[:, bass.ts(c, CHUNK)], scalar1=th, scalar2=0.0,
                        op0=Alu.subtract, op1=Alu.max,
                    )
                uv = u0[:].rearrange("p (f w j) -> p f w j", w=16, j=4)
                for w in range(16):
                    nc.tensor.matmul(
                        pc[:, c, 0:100].rearrange("p (f j) -> p f j", j=4),
                        wt[:, bass.ts(w, 128)],
                        uv[:, :, w, :],
                        start=(w == 0),
                        stop=(w == 15),
                    )
            # cnt [128, 500]: sum the 4 in-group columns out of PSUM
            cnt = shared[:, 0:WF]
            nc.vector.tensor_reduce(
                cnt,
                pc[:, :, 0:100].rearrange("p c (f j) -> p c f j", j=4),
                axis=Ax.X,
                op=Alu.add,
            )
            if STAGE < 3:
                nc.vector.tensor_reduce(dd, cnt, axis=Ax.X, op=Alu.add)
                nc.gpsimd.dma_start(out[bass.ts(b, 128)], dd)
                continue
            # candidate mask, wrapped gid+1 values, per-partition ranks
            maskv = shared[:, 512 : 512 + WF]
            cum = shared[:, 1024 : 1024 + WF]
            nc.vector.tensor_scalar(out=maskv, in0=cnt, scalar1=0.0, scalar2=None, op0=Alu.is_gt)
            nc.vector.scalar_tensor_tensor(
                out=v_i16[:, 0:WF], in0=maskv, scalar=1.0, in1=iot[:],
                op0=Alu.mult, op1=Alu.mult,
            )
            nc.vector.tensor_tensor_scan(
                out=cum, data0=maskv, data1=maskv, initial=0.0,
                op0=Alu.add, op1=Alu.bypass,
            )
            nc.vector.tensor_tensor(out=cum, in0=cum, in1=maskv, op=Alu.mult)
            nc.vector.scalar_tensor_tensor(
                out=cum, in0=cum, scalar=float(S4) + 0.5, in1=cum,
                op0=Alu.is_le, op1=Alu.mult,
            )
            nc.vector.tensor_scalar(out=rank[:, 0:WF], in0=cum, scalar1=-1.0, scalar2=None, op0=Alu.add)
            # compact gid list per partition (values are gid, pads 0 after scatter)
            nc.gpsimd.local_scatter(
                vcomp[:, 0:S4],
                v_i16[:, 0:WF],
                rank[:, 0:WF],
                channels=128,
                num_elems=S4,
                num_idxs=WF,
            )
            # group idx = (gid+1) - 1; scatter pads (0) map to sentinel group 8002
            nc.vector.tensor_scalar(
                out=bneg[:, 0:S4], in0=vcomp[:, 0:S4], scalar1=0.5, scalar2=8003.0,
                op0=Alu.is_lt, op1=Alu.mult,
            )
            nc.vector.scalar_tensor_tensor(
                out=bneg[:, 0:S4], in0=vcomp[:, 0:S4], scalar=1.0, in1=bneg[:, 0:S4],
                op0=Alu.mult, op1=Alu.add,
            )
            gidx = idxu.bitcast(i16)
            nc.vector.tensor_scalar(out=gidx[:, 0:S4], in0=bneg[:, 0:S4], scalar1=-1.0, scalar2=None, op0=Alu.add)
            if STAGE < 4:
                nc.vector.tensor_reduce(dd, bneg[:, 0:S4], axis=Ax.X, op=Alu.add)
                nc.gpsimd.dma_start(out[bass.ts(b, 128)], dd)
                continue
            nc.gpsimd.ap_gather(
                cand[:].rearrange("p (a d) -> p a d", d=4),
                xt[:].rearrange("p (a d) -> p a d", d=4),
                gidx[:, 0:S4],
                channels=128,
                num_elems=DW // 4,
                d=4,
                num_idxs=KU,
            )

            if STAGE < 5:
                nc.vector.tensor_reduce(dd, cand[:], axis=Ax.X, op=Alu.add)
                nc.gpsimd.dma_start(out[bass.ts(b, 128)], dd)
                continue
            # ---- exact Newton + final stats on the compact buffer ----
            for _ in range(EXACT_ITERS):
                nc.scalar.activation(wk[:], cand[:], Act.Relu, bias=nu, scale=1.0, accum_out=S1)
                nc.scalar.activation(wk[:], wk[:], Act.Square, accum_out=S2)
                nc.vector.reciprocal(r1, S1)
                nc.vector.tensor_scalar(
                    out=dd, in0=S2, scalar1=-4.0, scalar2=0.5, op0=Alu.add, op1=Alu.mult
                )
                nc.vector.scalar_tensor_tensor(
                    out=th, in0=dd, scalar=r1, in1=th, op0=Alu.mult, op1=Alu.add
                )
                nc.vector.tensor_scalar(out=nu, in0=th, scalar1=-1.0, scalar2=None, op0=Alu.mult)
            nc.vector.tensor_scalar(
                out=cand[:], in0=cand[:], scalar1=th, scalar2=0.0,
                op0=Alu.subtract, op1=Alu.max,
            )
            nc.scalar.activation(wk[:], cand[:], Act.Square, accum_out=S2f)
            nc.vector.tensor_tensor(out=wk[:], in0=wk[:], in1=cand[:], op=Alu.mult)
            nc.vector.tensor_reduce(S3f, wk[:], axis=Ax.X, op=Alu.add)
            # loss = 4/3 + S3f/12 + th*S2f/4 - xt
            nc.vector.scalar_tensor_tensor(
                out=ta, in0=S2f, scalar=0.25, in1=th, op0=Alu.mult, op1=Alu.mult
            )
            nc.vector.scalar_tensor_tensor(
                out=tb_, in0=S3f, scalar=1.0 / 12.0, in1=ta, op0=Alu.mult, op1=Alu.add
            )
            nc.vector.scalar_tensor_tensor(
                out=lo, in0=tb_, scalar=4.0 / 3.0, in1=xtg, op0=Alu.add, op1=Alu.subtract
            )
            nc.gpsimd.dma_start(out[bass.ts(b, 128)], lo)

    nc.compile()
    return nc


def get_nc():
    if "nc" not in _nc_cache:
        _nc_cache["nc"] = _build_nc()
    return _nc_cache["nc"]


def make_in_maps(X, target):
    import ml_dtypes

    X = np.ascontiguousarray(np.asarray(X, dtype=np.float32))
    target = np.asarray(target).astype(np.int64)

    # wrapped gid+1 iota: iot[p, f] = 16*f + (p % 16) + 1
    pp, ff = np.meshgrid(np.arange(128), np.arange(WF), indexing="ij")
    iot = (16 * ff + (pp % 16) + 1).astype(np.float32)
    # residue-selection matrices: wsel[p, w, n] = 1 if n == 16*(p//16) + w
    wsel = np.zeros((128, 16, 128), np.float32)
    for w in range(16):
        for p in range(128):
            wsel[p, w, 16 * (p // 16) + w] = 1.0
    wsel = wsel.reshape(128, 16 * 128).astype(ml_dtypes.bfloat16)

    in_maps = []
    for k in range(N_CORES):
        Xk = X[k * ROWS : (k + 1) * ROWS]
        tk = target[k * ROWS : (k + 1) * ROWS]
        ohk = np.zeros((ROWS, 64), np.float32)
        ohk[np.arange(ROWS), (tk % 64).astype(np.int64)] = 1.0
        tblk = np.zeros((128, 16), np.int16)
        for c in range(4):
            rows = np.arange(64)
            vals = (rows * (V // 64) + (tk[64 * c + rows] // 64)).astype(np.int16)
            w = np.zeros((16, 4), np.int16)
            w[rows % 16, rows // 16] = vals
            tblk[:, 4 * c : 4 * (c + 1)] = np.tile(w, (8, 1))
        in_maps.append({"x": Xk, "oh": ohk, "tbl": tblk, "iot": iot, "wsel": wsel})
    return in_maps


def kernel(X, target):
    from concourse.bass_utils import run_bass_kernel_spmd

    nc = get_nc()
    in_maps = make_in_maps(X, target)
    res = run_bass_kernel_spmd(nc, in_maps, core_ids=list(range(N_CORES)))
    loss = np.concatenate([r["loss"] for r in res.results]).astype(np.float32)
    return loss

